# revision 1
# baseline (speedup 1.0000x reference)
"""ARMA GNN (3 stacks, 4 layers, F=1) on 8 TRN2 NeuronCores.

Design:
  - Edges sharded by destination range across the 8 cores; the [N,K] node
    table is all-gathered between layers (no all-reduce needed).
  - Host precomputes pure STRUCTURE (canonical degree-bucketed node order,
    ELL layout [rows, 3, R] per partition, per-(slice, window) gather and
    placement index streams).  All VALUES (degrees, rsqrt, activations) are
    computed on device.
  - Per layer: bf16 table [pos, 4] staged slice-by-slice (broadcast to all
    128 partitions); ap_gather (d=4) fetches per-edge source values;
    local_scatter places them into 2046-element ELL windows; DMA-accumulate
    assembles the ELL in DRAM; dense multiply by edge weight + reduction
    gives the aggregates.
  - norm_e = dinv[row]*w_e*dinv[col]: dinv[row] is folded into table values
    at production, dinv[col] into the post-reduce scale, so the per-slot
    coefficient is just the (host-reordered) edge weight.
"""

import sys, types
sys.path.insert(0, '/opt/trn_rl_repo')

import numpy as np

K = 3
T = 4
NCORES = 8
P = 128
WIN = 2046
BUCKETS = [8, 16, 24, 32, 40, 48, 64, 96, 128, 256, 1024]
NSL = 80          # table slices (= gather passes per layer)
WG = 6            # windows per ap_gather call


def _install_ntff_hook():
    try:
        import trn_agent_boot.trn_boot as tb
        hook = tb._ntff_profile_via_ctypes('/opt/axon/libaxon_pjrt.so')
        mod = types.ModuleType('antenv.axon_hooks')
        mod.get_axon_ntff_profile_hook = lambda: hook
        mod.set_axon_ntff_profile_hook = lambda h: None
        sys.modules['antenv.axon_hooks'] = mod
    except Exception:
        pass


# ---------------------------------------------------------------------------
# Host-side structure compilation
# ---------------------------------------------------------------------------

def compile_structure(edge_index, n_nodes):
    row = np.asarray(edge_index[0], dtype=np.int64)
    col = np.asarray(edge_index[1], dtype=np.int64)
    NDST = n_nodes // NCORES

    deg = np.bincount(col, minlength=n_nodes)
    bucket_of = np.searchsorted(BUCKETS, deg)
    assert deg.max() <= BUCKETS[-1]

    nb_pad = np.zeros(len(BUCKETS), dtype=np.int64)
    per_nc_counts = []
    for i in range(NCORES):
        cnt = np.bincount(bucket_of[i * NDST:(i + 1) * NDST],
                          minlength=len(BUCKETS))
        per_nc_counts.append(cnt)
        nb_pad = np.maximum(nb_pad, -(-cnt // P))
    rows_pp = int(nb_pad.sum())
    BLOCK = rows_pp * P
    NPAD = BLOCK * NCORES

    perm = np.full(NPAD, -1, dtype=np.int64)
    inv = np.full(n_nodes, -1, dtype=np.int64)
    row_bucket = np.concatenate([np.full(int(nb_pad[b]), b, dtype=np.int64)
                                 for b in range(len(BUCKETS))])
    for i in range(NCORES):
        nodes = np.arange(i * NDST, (i + 1) * NDST)
        order = np.argsort(bucket_of[nodes], kind='stable')
        sorted_nodes = nodes[order]
        pos_in_block = np.empty(NDST, dtype=np.int64)
        off_rows, start = 0, 0
        for b in range(len(BUCKETS)):
            c = int(per_nc_counts[i][b])
            pos_in_block[start:start + c] = off_rows * P + np.arange(c)
            off_rows += int(nb_pad[b])
            start += c
        gpos = i * BLOCK + pos_in_block
        perm[gpos] = sorted_nodes
        inv[sorted_nodes] = gpos

    R_of_row = np.array([BUCKETS[int(b)] for b in row_bucket], dtype=np.int64)
    base3 = np.concatenate([[0], np.cumsum(3 * R_of_row)])
    base1 = np.concatenate([[0], np.cumsum(R_of_row)])
    ELL_ELEMS = int(base3[-1])

    win_rows_l, win_elems_l = [], []
    r = 0
    while r < rows_pp:
        b = row_bucket[r]
        cap = max(1, WIN // int(3 * R_of_row[r]))
        r2 = r
        while r2 < rows_pp and row_bucket[r2] == b and r2 - r < cap:
            r2 += 1
        win_rows_l.append(r)
        win_elems_l.append(int(base3[r]))
        r = r2
    NW = len(win_rows_l)
    win_rows = np.array(win_rows_l + [rows_pp], dtype=np.int64)
    win_elems = np.array(win_elems_l + [ELL_ELEMS], dtype=np.int64)
    win_of_row = np.repeat(np.arange(NW), np.diff(win_rows))

    S = 2 * (-(-NPAD // (NSL * 2)))
    assert NSL * S >= NPAD and S * 4 * 2 // 4 <= 32768 and S <= 16384

    nc_edges = []
    for i in range(NCORES):
        m = (col >= i * NDST) & (col < (i + 1) * NDST)
        er, ec = row[m], col[m]
        dpos = inv[ec] - i * BLOCK
        spos = inv[er]
        nc_edges.append(dict(q=dpos % P, r=dpos // P, sl=spos // S,
                             off=spos % S, g=(dpos % P) // 16,
                             w=win_of_row[dpos // P], eidx=np.nonzero(m)[0]))

    cellcnt = np.zeros((NCORES, NSL * NW * 8), dtype=np.int64)
    for i in range(NCORES):
        d = nc_edges[i]
        key = (d['sl'] * NW + d['w']) * 8 + d['g']
        cellcnt[i] = np.bincount(key, minlength=NSL * NW * 8)
    CNT = cellcnt.max(axis=0).reshape(NSL, NW, 8).max(axis=2)
    CNT = ((CNT + 3) // 4) * 4

    NWG = -(-NW // WG)
    # pad last window of each wgroup so each call's num_idxs % 16 == 0
    for j in range(NSL):
        for wg in range(NWG):
            ws = list(range(wg * WG, min((wg + 1) * WG, NW)))
            rem = int(CNT[j, ws].sum()) % 16
            if rem:
                CNT[j, ws[-1]] += 16 - rem
    call_off = np.zeros((NSL, NW), dtype=np.int64)
    call_len = np.zeros((NSL, NWG), dtype=np.int64)
    seg_off = np.zeros((NSL, NWG), dtype=np.int64)
    GIDX_L = 0
    for j in range(NSL):
        for wg in range(NWG):
            ws = np.arange(wg * WG, min((wg + 1) * WG, NW))
            offs = np.concatenate([[0], np.cumsum(CNT[j, ws])])
            call_off[j, ws] = offs[:-1]
            call_len[j, wg] = offs[-1]
            seg_off[j, wg] = GIDX_L
            GIDX_L += int(offs[-1]) // 16
    pseg_off = np.zeros((NSL, NW), dtype=np.int64)
    PIDX_L = 0
    for j in range(NSL):
        for w in range(NW):
            pseg_off[j, w] = PIDX_L
            PIDX_L += int(CNT[j, w]) * 4

    return dict(n_nodes=n_nodes, NDST=NDST, rows_pp=rows_pp, BLOCK=BLOCK,
                NPAD=NPAD, S=S, NW=NW, NWG=NWG, CNT=CNT, call_off=call_off,
                call_len=call_len, seg_off=seg_off, pseg_off=pseg_off,
                GIDX_L=GIDX_L, PIDX_L=PIDX_L, ELL_ELEMS=ELL_ELEMS,
                win_rows=win_rows, win_elems=win_elems, base3=base3,
                base1=base1, R_of_row=R_of_row, row_bucket=row_bucket,
                perm=perm, inv=inv, nc_edges=nc_edges)


def build_inputs_per_nc(st, edge_weight, x):
    NW = st['NW']
    CNT, call_off, seg_off, pseg_off = (st['CNT'], st['call_off'],
                                        st['seg_off'], st['pseg_off'])
    base3, R_of_row = st['base3'], st['R_of_row']
    win_elems = st['win_elems']
    rows_pp, BLOCK = st['rows_pp'], st['BLOCK']
    in_maps = []
    for i in range(NCORES):
        d = st['nc_edges'][i]
        q, r_, sl, off, g, w_ = (d['q'], d['r'], d['sl'], d['off'],
                                 d['g'], d['w'])
        ew = np.asarray(edge_weight, np.float32)[d['eidx']]
        ne = len(q)
        # slot rank t within each dst node
        dkey = r_ * P + q
        order = np.argsort(dkey, kind='stable')
        t = np.empty(ne, dtype=np.int64)
        sk = dkey[order]
        starts = np.concatenate([[0], np.nonzero(np.diff(sk))[0] + 1])
        runlen = np.diff(np.concatenate([starts, [ne]]))
        t[order] = np.arange(ne) - np.repeat(starts, runlen)
        # rank within (slice, window, group) cell
        ckey = (sl * NW + w_) * 8 + g
        corder = np.argsort(ckey, kind='stable')
        ck = ckey[corder]
        cst = np.concatenate([[0], np.nonzero(np.diff(ck))[0] + 1])
        crl = np.diff(np.concatenate([cst, [ne]]))
        cpos = np.empty(ne, dtype=np.int64)
        cpos[corder] = np.arange(ne) - np.repeat(cst, crl)

        jj = call_off[sl, w_] + cpos
        gidx = np.zeros((P, st['GIDX_L']), dtype=np.int16)
        wg = w_ // WG
        gidx[16 * g + jj % 16, seg_off[sl, wg] + jj // 16] = \
            off.astype(np.int16)

        pidx = np.full((P, st['PIDX_L']), -1, dtype=np.int16)
        elem0 = base3[r_] + t - win_elems[w_]
        Rr = R_of_row[r_]
        for s in range(K):
            pidx[q, pseg_off[sl, w_] + 4 * cpos + s] = \
                (elem0 + s * Rr).astype(np.int16)

        well3 = np.zeros((P, st['ELL_ELEMS']), dtype=np.float32)
        for s in range(K):
            well3[q, base3[r_] + s * Rr + t] = ew

        xd = np.zeros((P, rows_pp), dtype=np.float32)
        gpos = st['perm'][i * BLOCK:(i + 1) * BLOCK]
        valid = gpos >= 0
        xflat = np.zeros(BLOCK, dtype=np.float32)
        xflat[valid] = np.asarray(x, np.float32).reshape(-1)[gpos[valid]]
        xd[:, :] = xflat.reshape(rows_pp, P).T
        in_maps.append(dict(gidx=gidx, pidx=pidx, well3=well3, xd=xd))
    return in_maps


# ---------------------------------------------------------------------------
# Device kernel
# ---------------------------------------------------------------------------

def build_kernel(st, n_params):
    import concourse.bass as bass
    import concourse.bacc as bacc
    import concourse.mybir as mybir
    import concourse.tile as tile

    f32, bf16, i16 = mybir.dt.float32, mybir.dt.bfloat16, mybir.dt.int16
    rows, NW, NWG, S = st['rows_pp'], st['NW'], st['NWG'], st['S']
    CNT, call_off, call_len, seg_off, pseg_off = (
        st['CNT'], st['call_off'], st['call_len'], st['seg_off'],
        st['pseg_off'])
    win_rows, win_elems = st['win_rows'], st['win_elems']
    base3 = st['base3']
    BLOCK, ELL_ELEMS = st['BLOCK'], st['ELL_ELEMS']
    NNI = max(int(call_len[j, wg]) for j in range(NSL) for wg in range(NWG))

    nc = bacc.Bacc("TRN2", target_bir_lowering=False, debug=False,
                   num_devices=NCORES)
    gidx_d = nc.dram_tensor("gidx", [P, st['GIDX_L']], i16,
                            kind="ExternalInput").ap()
    pidx_d = nc.dram_tensor("pidx", [P, st['PIDX_L']], i16,
                            kind="ExternalInput").ap()
    well3_d = nc.dram_tensor("well3", [P, ELL_ELEMS], f32,
                             kind="ExternalInput").ap()
    xd_d = nc.dram_tensor("xd", [P, rows], f32, kind="ExternalInput").ap()
    par_d = nc.dram_tensor("par", [P, n_params], f32,
                           kind="ExternalInput").ap()
    out_d = nc.dram_tensor("out", [P, rows], f32, kind="ExternalOutput").ap()

    tbl = nc.dram_tensor("tbl", [NSL * S * 4], bf16)
    agin = nc.dram_tensor("agin", [BLOCK * 4], bf16)
    agout = nc.dram_tensor("agout", [NCORES * BLOCK * 4], bf16,
                           addr_space="Shared")
    with tile.TileContext(nc) as tc:
        with (
            tc.tile_pool(name="dpool", bufs=1, space="DRAM") as dpool,
            tc.tile_pool(name="big", bufs=1) as big,
            tc.tile_pool(name="sb", bufs=1) as sb,
            tc.tile_pool(name="sm", bufs=1) as sm,
            tc.tile_pool(name="dbl", bufs=2) as dbl,
        ):
            ell_t = dpool.tile([P, ELL_ELEMS], bf16, tag="ell")
            slice_t = big.tile([P, S * 4], bf16, tag="slice")
            act = big.tile([P, rows * K], f32, tag="act")
            dinv = big.tile([P, rows], f32, tag="dinv")
            xdt = big.tile([P, rows], f32, tag="xd")
            part = big.tile([P, n_params], f32, tag="par")
            nc.sync.dma_start(out=xdt[:], in_=xd_d[:])
            nc.sync.dma_start(out=part[:], in_=par_d[:])

            # ---- degree + dinv, per window (s=0 plane of well3) ----
            for w in range(NW):
                a, b = int(win_rows[w]), int(win_rows[w + 1])
                Rb = int(st['R_of_row'][a])
                nr = b - a
                wv = sm.tile([P, WIN], f32, tag="wv")
                nc.sync.dma_start(
                    out=wv[:, :nr * 3 * Rb],
                    in_=well3_d[:, int(win_elems[w]):int(win_elems[w + 1])])
                nc.vector.tensor_reduce(
                    out=dinv[:, a:b],
                    in_=wv[:, :nr * 3 * Rb].rearrange(
                        "p (r s t) -> p r s t", s=3, t=Rb)[:, :, 0, :],
                    axis=mybir.AxisListType.X, op=mybir.AluOpType.add)
            mask = sb.tile([P, rows], f32, tag="mask")
            nc.vector.tensor_scalar(out=mask[:], in0=dinv[:], scalar1=0.0,
                                    scalar2=None, op0=mybir.AluOpType.is_gt)
            nc.vector.tensor_scalar(out=dinv[:], in0=dinv[:], scalar1=1e-30,
                                    scalar2=None, op0=mybir.AluOpType.add)
            degt = sb.tile([P, rows], f32, tag="degt")
            nc.vector.tensor_copy(out=degt[:], in_=dinv[:])
            nc.scalar.activation(out=dinv[:], in_=dinv[:],
                                 func=mybir.ActivationFunctionType.Sqrt)
            nc.vector.reciprocal(out=dinv[:], in_=dinv[:])
            # Newton refinement: r <- r*(1.5 - 0.5*deg*r^2) (fixes LUT error)
            nwt = sb.tile([P, rows], f32, tag="nwt")
            nc.vector.tensor_tensor(out=nwt[:], in0=dinv[:], in1=dinv[:],
                                    op=mybir.AluOpType.mult)
            nc.vector.tensor_tensor(out=nwt[:], in0=nwt[:], in1=degt[:],
                                    op=mybir.AluOpType.mult)
            nc.vector.tensor_scalar(out=nwt[:], in0=nwt[:], scalar1=-0.5,
                                    scalar2=1.5, op0=mybir.AluOpType.mult,
                                    op1=mybir.AluOpType.add)
            nc.vector.tensor_tensor(out=dinv[:], in0=dinv[:], in1=nwt[:],
                                    op=mybir.AluOpType.mult)
            nc.vector.tensor_tensor(out=dinv[:], in0=dinv[:], in1=mask[:],
                                    op=mybir.AluOpType.mult)

            zt = sb.tile([P, WIN], bf16, tag="zt")
            nc.vector.memset(zt[:], 0.0)

            for t in range(T):
                # 1) produce U'[pos, k] = prev_k * dinv * W_t[k]
                up = sb.tile([P, rows * 4], bf16, tag="up")
                for k in range(K):
                    src = xdt[:] if t == 0 else \
                        act[:].rearrange("p (r k) -> p r k", k=K)[:, :, k]
                    tmp = sm.tile([P, rows], f32, tag="tmp")
                    nc.vector.tensor_tensor(out=tmp[:], in0=src,
                                            in1=dinv[:],
                                            op=mybir.AluOpType.mult)
                    nc.vector.tensor_scalar(
                        out=up[:].rearrange("p (r f) -> p r f", f=4)[:, :, k],
                        in0=tmp[:], scalar1=part[:, t * K + k:t * K + k + 1],
                        scalar2=None, op0=mybir.AluOpType.mult)
                nc.sync.dma_start(out=agin.ap(), in_=up[:])
                nc.gpsimd.collective_compute(
                    "AllGather", mybir.AluOpType.bypass,
                    replica_groups=[list(range(NCORES))],
                    ins=[agin.ap().opt()], outs=[agout.ap().opt()])
                nc.sync.dma_start(out=tbl.ap()[:NCORES * BLOCK * 4],
                                  in_=agout.ap())

                # zero the ELL accumulator
                for w in range(NW):
                    nc.sync.dma_start(
                        out=ell_t[:, int(win_elems[w]):
                                  int(win_elems[w + 1])],
                        in_=zt[:, :int(win_elems[w + 1] - win_elems[w])])

                # 2) gather + place + accumulate
                for j in range(NSL):
                    nc.sync.dma_start(
                        out=slice_t[:],
                        in_=tbl.ap()[j * S * 4:(j + 1) * S * 4]
                        .rearrange("(o x) -> o x", o=1)
                        .to_broadcast([P, S * 4]))
                    for wg in range(NWG):
                        L = int(call_len[j, wg])
                        if L == 0:
                            continue
                        gi = dbl.tile([P, max(NNI // 16, 16)], i16, tag="gi")
                        nc.sync.dma_start(
                            out=gi[:, :L // 16],
                            in_=gidx_d[:, int(seg_off[j, wg]):
                                       int(seg_off[j, wg]) + L // 16])
                        go = sm.tile([P, NNI * 4], bf16, tag="go")
                        nc.gpsimd.ap_gather(
                            out_ap=go[:, :L * 4].rearrange(
                                "p (n d) -> p n d", d=4),
                            in_ap=slice_t[:].rearrange(
                                "p (n d) -> p n d", d=4),
                            idxs_ap=gi[:, :L // 16], channels=P,
                            num_elems=S, d=4, num_idxs=L)
                        pi = dbl.tile([P, NNI * 4], i16, tag="pi")
                        w0, w1 = wg * WG, min(wg * WG + WG, NW)
                        p0 = int(pseg_off[j, w0])
                        p1 = int(pseg_off[j, w1 - 1] + CNT[j, w1 - 1] * 4)
                        nc.sync.dma_start(out=pi[:, :p1 - p0],
                                          in_=pidx_d[:, p0:p1])
                        for w in range(w0, w1):
                            cw = int(CNT[j, w])
                            if cw == 0:
                                continue
                            wel = int(win_elems[w + 1] - win_elems[w])
                            wt2 = sm.tile([P, WIN], bf16, tag="wt2")
                            doff = int(call_off[j, w]) * 4
                            poff = int(pseg_off[j, w]) - p0
                            nc.gpsimd.local_scatter(
                                out_ap=wt2[:, :wel],
                                data_ap=go[:, doff:doff + cw * 4],
                                idxs_ap=pi[:, poff:poff + cw * 4],
                                channels=P, num_elems=wel,
                                num_idxs=cw * 4)
                            nc.gpsimd.dma_start(
                                out=ell_t[:, int(win_elems[w]):
                                          int(win_elems[w + 1])],
                                in_=wt2[:, :wel],
                                accum_op=mybir.AluOpType.add)

                # 3) G = sum_R (ell * w);  4) act = relu(dinv*G + x*V + b)
                for w in range(NW):
                    a, b = int(win_rows[w]), int(win_rows[w + 1])
                    Rb = int(st['R_of_row'][a])
                    nr = b - a
                    et = sm.tile([P, WIN], bf16, tag="et")
                    nc.sync.dma_start(
                        out=et[:, :nr * 3 * Rb],
                        in_=ell_t[:, int(win_elems[w]):
                                  int(win_elems[w + 1])])
                    wv = sm.tile([P, WIN], f32, tag="wv")
                    nc.sync.dma_start(
                        out=wv[:, :nr * 3 * Rb],
                        in_=well3_d[:, int(win_elems[w]):
                                    int(win_elems[w + 1])])
                    pr = sm.tile([P, WIN], f32, tag="pr")
                    nc.vector.tensor_tensor(
                        out=pr[:, :nr * 3 * Rb], in0=et[:, :nr * 3 * Rb],
                        in1=wv[:, :nr * 3 * Rb], op=mybir.AluOpType.mult)
                    nc.vector.tensor_reduce(
                        out=act[:].rearrange("p (r k) -> p r k", k=K)
                        [:, a:b, :],
                        in_=pr[:, :nr * 3 * Rb].rearrange(
                            "p (r s t) -> p r s t", s=3, t=Rb),
                        axis=mybir.AxisListType.X, op=mybir.AluOpType.add)
                for k in range(K):
                    ak = act[:].rearrange("p (r k) -> p r k", k=K)[:, :, k]
                    nc.vector.tensor_tensor(out=ak, in0=ak, in1=dinv[:],
                                            op=mybir.AluOpType.mult)
                    tmp = sm.tile([P, rows], f32, tag="tmp")
                    c0 = T * K + t * K + k
                    nc.vector.tensor_scalar(
                        out=tmp[:], in0=xdt[:],
                        scalar1=part[:, c0:c0 + 1], scalar2=None,
                        op0=mybir.AluOpType.mult)
                    nc.vector.tensor_tensor(out=ak, in0=ak, in1=tmp[:],
                                            op=mybir.AluOpType.add)
                    c1 = 2 * T * K + t * K + k
                    nc.vector.tensor_scalar(
                        out=ak, in0=ak, scalar1=part[:, c1:c1 + 1],
                        scalar2=0.0, op0=mybir.AluOpType.add,
                        op1=mybir.AluOpType.max)

            fin = sb.tile([P, rows], f32, tag="fin")
            nc.vector.tensor_reduce(
                out=fin[:], in_=act[:].rearrange("p (r k) -> p r k", k=K),
                axis=mybir.AxisListType.X, op=mybir.AluOpType.add)
            c2 = 3 * T * K
            nc.vector.tensor_scalar(out=fin[:], in0=fin[:],
                                    scalar1=part[:, c2:c2 + 1], scalar2=None,
                                    op0=mybir.AluOpType.mult)
            nc.sync.dma_start(out=out_d[:], in_=fin[:])

    nc.finalize()
    from concourse.bass_interp import get_hw_module
    nc.m = get_hw_module(nc.m)
    return nc


# ---------------------------------------------------------------------------
# Entry point
# ---------------------------------------------------------------------------

def kernel(x, edge_index, edge_weight, init_weight, weight, root_weight,
           bias, lin_w, lin_b):
    _install_ntff_hook()
    from concourse.bass_utils import run_bass_kernel_spmd

    x = np.asarray(x, dtype=np.float32)
    n_nodes = x.shape[0]
    st = compile_structure(edge_index, n_nodes)
    in_maps_h = build_inputs_per_nc(st, edge_weight, x)

    Wt = np.zeros((T, K), np.float32)
    Wt[0] = np.asarray(init_weight, np.float32).reshape(K)
    for t in range(1, T):
        Wt[t] = np.asarray(weight, np.float32)[t - 1].reshape(K)
    rw = np.asarray(root_weight, np.float32).reshape(T, K)
    bi = np.asarray(bias, np.float32).reshape(T, K)
    pvec = np.concatenate([Wt.reshape(-1), rw.reshape(-1), bi.reshape(-1),
                           [float(np.asarray(lin_w).reshape(-1)[0]) / K,
                            float(np.asarray(lin_b).reshape(-1)[0])]])
    params_np = np.tile(pvec[None, :], (P, 1)).astype(np.float32)

    nc = build_kernel(st, params_np.shape[1])
    in_maps = []
    for i in range(NCORES):
        m = in_maps_h[i]
        in_maps.append({"gidx": m['gidx'], "pidx": m['pidx'],
                        "well3": m['well3'], "xd": m['xd'],
                        "par": params_np})
    import os
    do_trace = os.environ.get("KERNEL_TRACE", "0") == "1"
    try:
        res = run_bass_kernel_spmd(nc, in_maps,
                                   core_ids=list(range(NCORES)),
                                   trace=do_trace)
    except Exception:
        res = run_bass_kernel_spmd(nc, in_maps,
                                   core_ids=list(range(NCORES)), trace=False)
    kernel._last_exec_ns = getattr(res, 'exec_time_ns', None)

    out = np.zeros(n_nodes, dtype=np.float32)
    BLOCK = st['BLOCK']
    for i in range(NCORES):
        flat = res.results[i]["out"].T.reshape(-1)
        gpos = st['perm'][i * BLOCK:(i + 1) * BLOCK]
        valid = gpos >= 0
        out[gpos[valid]] = flat[valid]
    out = out + float(np.asarray(lin_b).reshape(-1)[0])
    out = 1.0 / (1.0 + np.exp(-out.astype(np.float64)))
    return out.reshape(n_nodes, 1).astype(np.float32)

